# revision 6
# baseline (speedup 1.0000x reference)
"""Trainium2 Bass kernel for the Flux_Kernels 5-point Dirichlet stencil.

out[i,j] = D*s0*(u[i-1,j] + u[i+1,j] + u[i,j-1] + u[i,j+1]) + 4*D*s1*u[i,j]
with out-of-range neighbors replaced by dirichlet_val[{0,1,2,3}].

Strategy: pad u with the Dirichlet constants into S [4098, 4098] on the host,
shard along rows: core k gets S[512k : 512k+514] (1-row halo each side baked
into the slab). On each core, tiles of 128 consecutive padded rows are
processed with partition p <-> padded row r0+p:
  - TensorE mm1: tridiagonal W.T @ tile[:, 1+cc:]  -> PSUM[p] = a*up + c*ctr
    + a*down centered at padded row r0+p (start=True)
  - TensorE mm2: (a*I).T @ tile[:, cc:] accumulated -> += a*left  (the PE
    absorbs the column shift via the rhs access-pattern offset)
  - VectorE: o[p] = (tile[p, 2+cc:] * a) + PSUM[p]  (fused
    scalar_tensor_tensor supplies the a*right term and evacuates PSUM)
  - output DMA stores partitions 1..126 -> 126 output rows per tile; the
    DMA absorbs the one-row shift that compute engines cannot express
    (rows 0 and 127 of each tile are incomplete and simply never stored).
Consecutive tiles overlap by 2 rows; all scalars (a = D*s0, c = 4*D*s1,
weight matrices) are computed on the host from the runtime inputs.
"""

import sys

import numpy as np

if "/opt/trn_rl_repo" not in sys.path:
    sys.path.insert(0, "/opt/trn_rl_repo")

NX, NY = 4096, 4096
N_CORES = 8
ROWS_PER_CORE = NX // N_CORES          # 512
SLAB_ROWS = ROWS_PER_CORE + 2          # 514
PAD_COLS = NY + 2                      # 4098
TILE_OUT = 126                         # output rows per full tile
FULL_TILES = ROWS_PER_CORE // TILE_OUT  # 4
LAST_OUT = ROWS_PER_CORE - FULL_TILES * TILE_OUT  # 8
LAST_IN = LAST_OUT + 2                 # 10
PSUM_CHUNK = 1024                      # free-dim columns per PSUM tile (2 banks)
MM_N = 512                             # matmul moving free dim (1 PSUM bank)
DMA_HALF = 2050                        # input DMA split: [0:2050), [2048:4098)

_CACHE: dict = {}


def _build_nc():
    import concourse.bass as bass
    import concourse.mybir as mybir
    from concourse import bacc
    from concourse.tile import TileContext

    f32 = mybir.dt.float32
    add = mybir.AluOpType.add
    mult = mybir.AluOpType.mult

    nc = bacc.Bacc(None, target_bir_lowering=False)
    s_in = nc.dram_tensor("s_in", (SLAB_ROWS, PAD_COLS), f32, kind="ExternalInput")
    w_main = nc.dram_tensor("w_main", (128, 128), f32, kind="ExternalInput")
    w_diag = nc.dram_tensor("w_diag", (128, 128), f32, kind="ExternalInput")
    w_last = nc.dram_tensor("w_last", (LAST_IN, LAST_IN), f32, kind="ExternalInput")
    w_dlast = nc.dram_tensor("w_dlast", (LAST_IN, LAST_IN), f32, kind="ExternalInput")
    coef = nc.dram_tensor("coef", (128, 1), f32, kind="ExternalInput")
    out = nc.dram_tensor("out", (ROWS_PER_CORE, NY), f32, kind="ExternalOutput")

    n_tiles = FULL_TILES + 1

    with TileContext(nc) as tc:
        with (
            tc.tile_pool(name="const", bufs=1) as cpool,
            tc.tile_pool(name="inp", bufs=3) as ipool,
            tc.tile_pool(name="op", bufs=2) as opool,
            tc.tile_pool(name="psum", bufs=4, space=bass.MemorySpace.PSUM) as ppool,
        ):
            # first input tile half goes out before anything else so the PE
            # can start as early as possible
            in_tiles = []
            in0 = ipool.tile([128, PAD_COLS], f32, tag="in")
            in_tiles.append(in0)
            nc.sync.dma_start(out=in0[:128, 0:DMA_HALF], in_=s_in[0:128, 0:DMA_HALF])

            w_t = cpool.tile([128, 128], f32)
            nc.sync.dma_start(out=w_t[:], in_=w_main[:])
            wd_t = cpool.tile([128, 128], f32)
            nc.sync.dma_start(out=wd_t[:], in_=w_diag[:])
            w5_t = cpool.tile([LAST_IN, LAST_IN], f32)
            nc.sync.dma_start(out=w5_t[:], in_=w_last[:])
            wd5_t = cpool.tile([LAST_IN, LAST_IN], f32)
            nc.sync.dma_start(out=wd5_t[:], in_=w_dlast[:])
            coef_t = cpool.tile([128, 1], f32)
            nc.sync.dma_start(out=coef_t[:], in_=coef[:])

            nc.sync.dma_start(
                out=in0[:128, 2048:PAD_COLS], in_=s_in[0:128, 2048:PAD_COLS]
            )

            for t in range(n_tiles):
                r0 = TILE_OUT * t
                ri = 128 if t < FULL_TILES else LAST_IN
                ro = TILE_OUT if t < FULL_TILES else LAST_OUT
                wt = w_t if t < FULL_TILES else w5_t
                wdt = wd_t if t < FULL_TILES else wd5_t

                if t == 0:
                    in_t = in_tiles[0]
                else:
                    in_t = ipool.tile([128, PAD_COLS], f32, tag="in")
                    nc.sync.dma_start(
                        out=in_t[:ri, 0:DMA_HALF], in_=s_in[r0 : r0 + ri, 0:DMA_HALF]
                    )
                    nc.sync.dma_start(
                        out=in_t[:ri, 2048:PAD_COLS],
                        in_=s_in[r0 : r0 + ri, 2048:PAD_COLS],
                    )

                o_t = opool.tile([128, NY], f32, tag="o")
                for h in range(NY // PSUM_CHUNK):
                    ps = ppool.tile([128, PSUM_CHUNK], f32, tag="ps")
                    for q in range(PSUM_CHUNK // MM_N):
                        cc = h * PSUM_CHUNK + q * MM_N
                        # a*up + c*ctr + a*down (rows via tridiagonal weights)
                        nc.tensor.matmul(
                            ps[:ri, q * MM_N : (q + 1) * MM_N],
                            wt[:ri, :ri],
                            in_t[:ri, 1 + cc : 1 + cc + MM_N],
                            start=True,
                            stop=False,
                        )
                        # += a*left (diagonal weights, column-shifted rhs)
                        nc.tensor.matmul(
                            ps[:ri, q * MM_N : (q + 1) * MM_N],
                            wdt[:ri, :ri],
                            in_t[:ri, cc : cc + MM_N],
                            start=False,
                            stop=True,
                        )
                    # o = a*right + psum; also evacuates PSUM to SBUF
                    nc.vector.scalar_tensor_tensor(
                        out=o_t[:ri, h * PSUM_CHUNK : (h + 1) * PSUM_CHUNK],
                        in0=in_t[:ri, 2 + h * PSUM_CHUNK : 2 + (h + 1) * PSUM_CHUNK],
                        scalar=coef_t[:ri],
                        in1=ps[:ri, :],
                        op0=mult,
                        op1=add,
                    )
                for g in range(2):
                    nc.sync.dma_start(
                        out=out[r0 : r0 + ro, g * 2048 : (g + 1) * 2048],
                        in_=o_t[1 : 1 + ro, g * 2048 : (g + 1) * 2048],
                    )

    nc.compile()
    return nc


def _get_nc():
    if "nc" not in _CACHE:
        _CACHE["nc"] = _build_nc()
    return _CACHE["nc"]


def _tridiag(n, a, c):
    w = np.zeros((n, n), dtype=np.float32)
    i = np.arange(n)
    w[i, i] = c
    w[i[:-1], i[1:]] = a  # k = m-1 (up neighbor)
    w[i[1:], i[:-1]] = a  # k = m+1 (down neighbor)
    return w


def _weight_inputs(a, c):
    return {
        "w_main": _tridiag(128, a, c),
        "w_diag": np.diag(np.full(128, a, np.float32)).astype(np.float32),
        "w_last": _tridiag(LAST_IN, a, c),
        "w_dlast": np.diag(np.full(LAST_IN, a, np.float32)).astype(np.float32),
        "coef": np.full((128, 1), a, dtype=np.float32),
    }


def kernel(u_main, u_coupled=None, D_eff=None, dirichlet_val=None, stencil=None,
           t=None, **_ignored):
    u = np.asarray(u_main, dtype=np.float32)
    assert u.shape == (NX, NY), u.shape
    D = float(np.asarray(D_eff).reshape(-1)[0])
    st = np.asarray(stencil).reshape(-1)
    s0, s1 = float(st[0]), float(st[1])
    dv = np.asarray(dirichlet_val, dtype=np.float32).reshape(-1)
    a = np.float32(D * s0)
    c = np.float32(4.0 * D * s1)

    S = np.empty((NX + 2, NY + 2), dtype=np.float32)
    S[1:-1, 1:-1] = u
    S[0, :] = dv[0]       # x- boundary (row 0 up-neighbor)
    S[-1, :] = dv[1]      # x+ boundary
    S[1:-1, 0] = dv[2]    # y- boundary
    S[1:-1, -1] = dv[3]   # y+ boundary

    in_maps = [
        {
            "s_in": np.ascontiguousarray(S[ROWS_PER_CORE * k : ROWS_PER_CORE * k + SLAB_ROWS]),
            **_weight_inputs(a, c),
        }
        for k in range(N_CORES)
    ]

    from concourse.bass_utils import run_bass_kernel_spmd

    res = run_bass_kernel_spmd(_get_nc(), in_maps, core_ids=list(range(N_CORES)))
    return np.concatenate([r["out"] for r in res.results], axis=0)


# revision 9
# speedup vs baseline: 1.0311x; 1.0311x over previous
"""Trainium2 Bass kernel for the Flux_Kernels 5-point Dirichlet stencil.

out[i,j] = D*s0*(u[i-1,j] + u[i+1,j] + u[i,j-1] + u[i,j+1]) + 4*D*s1*u[i,j]
with out-of-range neighbors replaced by dirichlet_val[{0,1,2,3}].

Strategy: pad u with the Dirichlet constants into S [4098, 4098] on the host,
shard along rows: core k gets S[512k : 512k+514] (1-row halo each side baked
into the slab). On each core, tiles of 128 consecutive padded rows are
processed with partition p <-> padded row r0+p:
  - TensorE mm1: tridiagonal W.T @ tile[:, 1+cc:]  -> PSUM[p] = a*up + c*ctr
    + a*down centered at padded row r0+p (start=True)
  - TensorE mm2: (a*I).T @ tile[:, cc:] accumulated -> += a*left  (the PE
    absorbs the column shift via the rhs access-pattern offset)
  - VectorE: o[p] = (tile[p, 2+cc:] * a) + PSUM[p]  (fused
    scalar_tensor_tensor supplies the a*right term and evacuates PSUM)
  - output DMA stores partitions 1..126 -> 126 output rows per tile; the
    DMA absorbs the one-row shift that compute engines cannot express
    (rows 0 and 127 of each tile are incomplete and simply never stored).
Consecutive tiles overlap by 2 rows; all scalars (a = D*s0, c = 4*D*s1,
weight matrices) are computed on the host from the runtime inputs.
"""

import sys

import numpy as np

if "/opt/trn_rl_repo" not in sys.path:
    sys.path.insert(0, "/opt/trn_rl_repo")

NX, NY = 4096, 4096
N_CORES = 8
ROWS_PER_CORE = NX // N_CORES          # 512
SLAB_ROWS = ROWS_PER_CORE + 2          # 514
PAD_COLS = NY + 2                      # 4098
TILE_OUT = 126                         # output rows per full tile
FULL_TILES = ROWS_PER_CORE // TILE_OUT  # 4
LAST_OUT = ROWS_PER_CORE - FULL_TILES * TILE_OUT  # 8
LAST_IN = LAST_OUT + 2                 # 10
PSUM_CHUNK = 1024                      # free-dim columns per PSUM tile (2 banks)
MM_N = 512                             # matmul moving free dim (1 PSUM bank)
DMA_HALF = 2050                        # input DMA split: [0:2050), [2048:4098)

_CACHE: dict = {}


def _build_nc():
    import concourse.bass as bass
    import concourse.mybir as mybir
    from concourse import bacc
    from concourse.tile import TileContext

    f32 = mybir.dt.float32
    add = mybir.AluOpType.add
    mult = mybir.AluOpType.mult

    nc = bacc.Bacc(None, target_bir_lowering=False)
    s_in = nc.dram_tensor("s_in", (SLAB_ROWS, PAD_COLS), f32, kind="ExternalInput")
    w_main = nc.dram_tensor("w_main", (128, 128), f32, kind="ExternalInput")
    w_diag = nc.dram_tensor("w_diag", (128, 128), f32, kind="ExternalInput")
    w_last = nc.dram_tensor("w_last", (LAST_IN, LAST_IN), f32, kind="ExternalInput")
    w_dlast = nc.dram_tensor("w_dlast", (LAST_IN, LAST_IN), f32, kind="ExternalInput")
    coef = nc.dram_tensor("coef", (128, 1), f32, kind="ExternalInput")
    out = nc.dram_tensor("out", (ROWS_PER_CORE, NY), f32, kind="ExternalOutput")

    n_tiles = FULL_TILES + 1

    with TileContext(nc) as tc:
        with (
            tc.tile_pool(name="const", bufs=1) as cpool,
            tc.tile_pool(name="inp", bufs=3) as ipool,
            tc.tile_pool(name="lrp", bufs=2) as lpool,
            tc.tile_pool(name="op", bufs=2) as opool,
            tc.tile_pool(name="psum", bufs=4, space=bass.MemorySpace.PSUM) as ppool,
        ):
            # first input tile quarter goes out before anything else so the
            # PE can start as early as possible
            in_tiles = []
            in0 = ipool.tile([128, PAD_COLS], f32, tag="in")
            in_tiles.append(in0)
            nc.sync.dma_start(out=in0[:128, 0:1026], in_=s_in[0:128, 0:1026])

            w_t = cpool.tile([128, 128], f32)
            nc.sync.dma_start(out=w_t[:], in_=w_main[:])
            wd_t = cpool.tile([128, 128], f32)
            nc.sync.dma_start(out=wd_t[:], in_=w_diag[:])
            w5_t = cpool.tile([LAST_IN, LAST_IN], f32)
            nc.sync.dma_start(out=w5_t[:], in_=w_last[:])
            wd5_t = cpool.tile([LAST_IN, LAST_IN], f32)
            nc.sync.dma_start(out=wd5_t[:], in_=w_dlast[:])
            coef_t = cpool.tile([128, 1], f32)
            nc.sync.dma_start(out=coef_t[:], in_=coef[:])

            for c0, c1 in ((1024, 2050), (2048, 3074), (3072, PAD_COLS)):
                nc.sync.dma_start(out=in0[:128, c0:c1], in_=s_in[0:128, c0:c1])

            n_chunks = NY // PSUM_CHUNK          # 4 psum tiles per row-tile
            pe_chunks = 3                        # chunks via PE-left path
            lr_c0 = pe_chunks * PSUM_CHUNK       # DVE-ADD path covers the rest

            for t in range(n_tiles):
                r0 = TILE_OUT * t
                ri = 128 if t < FULL_TILES else LAST_IN
                ro = TILE_OUT if t < FULL_TILES else LAST_OUT
                wt = w_t if t < FULL_TILES else w5_t
                wdt = wd_t if t < FULL_TILES else wd5_t

                if t == 0:
                    in_t = in_tiles[0]
                else:
                    in_t = ipool.tile([128, PAD_COLS], f32, tag="in")
                    nc.sync.dma_start(
                        out=in_t[:ri, 0:DMA_HALF], in_=s_in[r0 : r0 + ri, 0:DMA_HALF]
                    )
                    nc.sync.dma_start(
                        out=in_t[:ri, 2048:PAD_COLS],
                        in_=s_in[r0 : r0 + ri, 2048:PAD_COLS],
                    )

                # left+right sums for the DVE-path columns [lr_c0, NY)
                lr_t = lpool.tile([128, NY - lr_c0], f32, tag="lr")
                nc.vector.tensor_add(
                    out=lr_t[:ri],
                    in0=in_t[:ri, lr_c0:NY],
                    in1=in_t[:ri, lr_c0 + 2 : NY + 2],
                )

                o_t = opool.tile([128, NY], f32, tag="o")
                for h in range(n_chunks):
                    pe_path = h < pe_chunks
                    ps = ppool.tile([128, PSUM_CHUNK], f32, tag="ps")
                    for q in range(PSUM_CHUNK // MM_N):
                        cc = h * PSUM_CHUNK + q * MM_N
                        # a*up + c*ctr + a*down (rows via tridiagonal weights)
                        nc.tensor.matmul(
                            ps[:ri, q * MM_N : (q + 1) * MM_N],
                            wt[:ri, :ri],
                            in_t[:ri, 1 + cc : 1 + cc + MM_N],
                            start=True,
                            stop=not pe_path,
                        )
                        if pe_path:
                            # += a*left (diagonal weights, column-shifted rhs)
                            nc.tensor.matmul(
                                ps[:ri, q * MM_N : (q + 1) * MM_N],
                                wdt[:ri, :ri],
                                in_t[:ri, cc : cc + MM_N],
                                start=False,
                                stop=True,
                            )
                    if pe_path:
                        # o = a*right + psum (right read straight from input)
                        in0 = in_t[:ri, 2 + h * PSUM_CHUNK : 2 + (h + 1) * PSUM_CHUNK]
                    else:
                        # o = a*(left+right) + psum
                        in0 = lr_t[:ri, h * PSUM_CHUNK - lr_c0 : (h + 1) * PSUM_CHUNK - lr_c0]
                    nc.vector.scalar_tensor_tensor(
                        out=o_t[:ri, h * PSUM_CHUNK : (h + 1) * PSUM_CHUNK],
                        in0=in0,
                        scalar=coef_t[:ri],
                        in1=ps[:ri, :],
                        op0=mult,
                        op1=add,
                    )
                    nc.sync.dma_start(
                        out=out[r0 : r0 + ro, h * PSUM_CHUNK : (h + 1) * PSUM_CHUNK],
                        in_=o_t[1 : 1 + ro, h * PSUM_CHUNK : (h + 1) * PSUM_CHUNK],
                    )

    nc.compile()
    return nc


def _get_nc():
    if "nc" not in _CACHE:
        _CACHE["nc"] = _build_nc()
    return _CACHE["nc"]


def _tridiag(n, a, c):
    w = np.zeros((n, n), dtype=np.float32)
    i = np.arange(n)
    w[i, i] = c
    w[i[:-1], i[1:]] = a  # k = m-1 (up neighbor)
    w[i[1:], i[:-1]] = a  # k = m+1 (down neighbor)
    return w


def _weight_inputs(a, c):
    return {
        "w_main": _tridiag(128, a, c),
        "w_diag": np.diag(np.full(128, a, np.float32)).astype(np.float32),
        "w_last": _tridiag(LAST_IN, a, c),
        "w_dlast": np.diag(np.full(LAST_IN, a, np.float32)).astype(np.float32),
        "coef": np.full((128, 1), a, dtype=np.float32),
    }


def kernel(u_main, u_coupled=None, D_eff=None, dirichlet_val=None, stencil=None,
           t=None, **_ignored):
    u = np.asarray(u_main, dtype=np.float32)
    assert u.shape == (NX, NY), u.shape
    D = float(np.asarray(D_eff).reshape(-1)[0])
    st = np.asarray(stencil).reshape(-1)
    s0, s1 = float(st[0]), float(st[1])
    dv = np.asarray(dirichlet_val, dtype=np.float32).reshape(-1)
    a = np.float32(D * s0)
    c = np.float32(4.0 * D * s1)

    S = np.empty((NX + 2, NY + 2), dtype=np.float32)
    S[1:-1, 1:-1] = u
    S[0, :] = dv[0]       # x- boundary (row 0 up-neighbor)
    S[-1, :] = dv[1]      # x+ boundary
    S[1:-1, 0] = dv[2]    # y- boundary
    S[1:-1, -1] = dv[3]   # y+ boundary

    in_maps = [
        {
            "s_in": np.ascontiguousarray(S[ROWS_PER_CORE * k : ROWS_PER_CORE * k + SLAB_ROWS]),
            **_weight_inputs(a, c),
        }
        for k in range(N_CORES)
    ]

    from concourse.bass_utils import run_bass_kernel_spmd

    res = run_bass_kernel_spmd(_get_nc(), in_maps, core_ids=list(range(N_CORES)))
    return np.concatenate([r["out"] for r in res.results], axis=0)


# revision 15
# speedup vs baseline: 1.1215x; 1.0876x over previous
"""Trainium2 Bass kernel for the Flux_Kernels 5-point Dirichlet stencil.

out[i,j] = D*s0*(u[i-1,j] + u[i+1,j] + u[i,j-1] + u[i,j+1]) + 4*D*s1*u[i,j]
with out-of-range neighbors replaced by dirichlet_val[{0,1,2,3}].

Strategy: pad u with the Dirichlet constants into S [4098, 4098] on the host,
shard along rows: core k gets S[512k : 512k+514] (1-row halo each side baked
into the slab). On each core, tiles of 128 consecutive padded rows are
processed with partition p <-> padded row r0+p:
  - TensorE mm1: tridiagonal W.T @ tile[:, 1+cc:]  -> PSUM[p] = a*up + c*ctr
    + a*down centered at padded row r0+p (start=True)
  - TensorE mm2: (a*I).T @ tile[:, cc:] accumulated -> += a*left  (the PE
    absorbs the column shift via the rhs access-pattern offset)
  - VectorE: o[p] = (tile[p, 2+cc:] * a) + PSUM[p]  (fused
    scalar_tensor_tensor supplies the a*right term and evacuates PSUM)
  - output DMA stores partitions 1..126 -> 126 output rows per tile; the
    DMA absorbs the one-row shift that compute engines cannot express
    (rows 0 and 127 of each tile are incomplete and simply never stored).
Consecutive tiles overlap by 2 rows; all scalars (a = D*s0, c = 4*D*s1,
weight matrices) are computed on the host from the runtime inputs.
"""

import sys

import numpy as np

if "/opt/trn_rl_repo" not in sys.path:
    sys.path.insert(0, "/opt/trn_rl_repo")

NX, NY = 4096, 4096
N_CORES = 8
ROWS_PER_CORE = NX // N_CORES          # 512
SLAB_ROWS = ROWS_PER_CORE + 2          # 514
PAD_COLS = NY + 2                      # 4098
TILE_OUT = 126                         # output rows per full tile
FULL_TILES = ROWS_PER_CORE // TILE_OUT  # 4
LAST_OUT = ROWS_PER_CORE - FULL_TILES * TILE_OUT  # 8
LAST_IN = LAST_OUT + 2                 # 10
PSUM_CHUNK = 2048                      # free-dim columns per PSUM tile (4 banks)
MM_N = 512                             # matmul moving free dim (1 PSUM bank)
DMA_HALF = 2050                        # input DMA split: [0:2050), [2048:4098)

_CACHE: dict = {}


def _build_nc():
    import concourse.bass as bass
    import concourse.mybir as mybir
    from concourse import bacc
    from concourse.tile import TileContext

    f32 = mybir.dt.float32
    add = mybir.AluOpType.add
    mult = mybir.AluOpType.mult

    nc = bacc.Bacc(None, target_bir_lowering=False)
    s_in = nc.dram_tensor("s_in", (SLAB_ROWS, PAD_COLS), f32, kind="ExternalInput")
    w_main = nc.dram_tensor("w_main", (128, 128), f32, kind="ExternalInput")
    w_last = nc.dram_tensor("w_last", (LAST_IN, LAST_IN), f32, kind="ExternalInput")
    coef = nc.dram_tensor("coef", (128, 1), f32, kind="ExternalInput")
    out = nc.dram_tensor("out", (ROWS_PER_CORE, NY), f32, kind="ExternalOutput")

    n_tiles = FULL_TILES + 1

    with TileContext(nc) as tc:
        with (
            tc.tile_pool(name="const", bufs=1) as cpool,
            tc.tile_pool(name="inp", bufs=3) as ipool,
            tc.tile_pool(name="lrp", bufs=2) as lpool,
            tc.tile_pool(name="op", bufs=2) as opool,
            tc.tile_pool(name="psum", bufs=2, space=bass.MemorySpace.PSUM) as ppool,
        ):
            # first input tile quarter goes out before anything else so the
            # PE can start as early as possible
            in_tiles = []
            in0 = ipool.tile([128, PAD_COLS], f32, tag="in")
            in_tiles.append(in0)
            nc.sync.dma_start(out=in0[:128, 0:1026], in_=s_in[0:128, 0:1026])

            w_t = cpool.tile([128, 128], f32)
            nc.sync.dma_start(out=w_t[:], in_=w_main[:])
            w5_t = cpool.tile([LAST_IN, LAST_IN], f32)
            nc.sync.dma_start(out=w5_t[:], in_=w_last[:])
            coef_t = cpool.tile([128, 1], f32)
            nc.sync.dma_start(out=coef_t[:], in_=coef[:])

            for c0, c1 in ((1024, 2050), (2048, 3074), (3072, PAD_COLS)):
                nc.sync.dma_start(out=in0[:128, c0:c1], in_=s_in[0:128, c0:c1])

            n_chunks = NY // PSUM_CHUNK          # psum tiles per row-tile

            for t in range(n_tiles):
                r0 = TILE_OUT * t
                ri = 128 if t < FULL_TILES else LAST_IN
                ro = TILE_OUT if t < FULL_TILES else LAST_OUT
                wt = w_t if t < FULL_TILES else w5_t

                if t == 0:
                    in_t = in_tiles[0]
                else:
                    in_t = ipool.tile([128, PAD_COLS], f32, tag="in")
                    nc.sync.dma_start(
                        out=in_t[:ri, 0:DMA_HALF], in_=s_in[r0 : r0 + ri, 0:DMA_HALF]
                    )
                    nc.sync.dma_start(
                        out=in_t[:ri, 2048:PAD_COLS],
                        in_=s_in[r0 : r0 + ri, 2048:PAD_COLS],
                    )

                # left+right sums (one whole-row DVE pass)
                lr_t = lpool.tile([128, NY], f32, tag="lr")
                nc.vector.tensor_add(
                    out=lr_t[:ri], in0=in_t[:ri, 0:NY], in1=in_t[:ri, 2 : NY + 2]
                )

                o_t = opool.tile([128, NY], f32, tag="o")
                for h in range(n_chunks):
                    ps = ppool.tile([128, PSUM_CHUNK], f32, tag="ps")
                    for q in range(PSUM_CHUNK // MM_N):
                        cc = h * PSUM_CHUNK + q * MM_N
                        # a*up + c*ctr + a*down (rows via tridiagonal weights)
                        nc.tensor.matmul(
                            ps[:ri, q * MM_N : (q + 1) * MM_N],
                            wt[:ri, :ri],
                            in_t[:ri, 1 + cc : 1 + cc + MM_N],
                            start=True,
                            stop=True,
                        )
                    # o = a*(left+right) + psum; also evacuates PSUM
                    nc.vector.scalar_tensor_tensor(
                        out=o_t[:ri, h * PSUM_CHUNK : (h + 1) * PSUM_CHUNK],
                        in0=lr_t[:ri, h * PSUM_CHUNK : (h + 1) * PSUM_CHUNK],
                        scalar=coef_t[:ri],
                        in1=ps[:ri, :],
                        op0=mult,
                        op1=add,
                    )
                    # stores go on the ACT HWDGE ring so they never
                    # head-of-line-block input prefetches on the SP ring
                    nc.scalar.dma_start(
                        out=out[r0 : r0 + ro, h * PSUM_CHUNK : (h + 1) * PSUM_CHUNK],
                        in_=o_t[1 : 1 + ro, h * PSUM_CHUNK : (h + 1) * PSUM_CHUNK],
                    )

    nc.compile()
    return nc


def _get_nc():
    if "nc" not in _CACHE:
        _CACHE["nc"] = _build_nc()
    return _CACHE["nc"]


def _tridiag(n, a, c):
    w = np.zeros((n, n), dtype=np.float32)
    i = np.arange(n)
    w[i, i] = c
    w[i[:-1], i[1:]] = a  # k = m-1 (up neighbor)
    w[i[1:], i[:-1]] = a  # k = m+1 (down neighbor)
    return w


def _weight_inputs(a, c):
    return {
        "w_main": _tridiag(128, a, c),
        "w_last": _tridiag(LAST_IN, a, c),
        "coef": np.full((128, 1), a, dtype=np.float32),
    }


def kernel(u_main, u_coupled=None, D_eff=None, dirichlet_val=None, stencil=None,
           t=None, **_ignored):
    u = np.asarray(u_main, dtype=np.float32)
    assert u.shape == (NX, NY), u.shape
    D = float(np.asarray(D_eff).reshape(-1)[0])
    st = np.asarray(stencil).reshape(-1)
    s0, s1 = float(st[0]), float(st[1])
    dv = np.asarray(dirichlet_val, dtype=np.float32).reshape(-1)
    a = np.float32(D * s0)
    c = np.float32(4.0 * D * s1)

    S = np.empty((NX + 2, NY + 2), dtype=np.float32)
    S[1:-1, 1:-1] = u
    S[0, :] = dv[0]       # x- boundary (row 0 up-neighbor)
    S[-1, :] = dv[1]      # x+ boundary
    S[1:-1, 0] = dv[2]    # y- boundary
    S[1:-1, -1] = dv[3]   # y+ boundary

    in_maps = [
        {
            "s_in": np.ascontiguousarray(S[ROWS_PER_CORE * k : ROWS_PER_CORE * k + SLAB_ROWS]),
            **_weight_inputs(a, c),
        }
        for k in range(N_CORES)
    ]

    from concourse.bass_utils import run_bass_kernel_spmd

    res = run_bass_kernel_spmd(_get_nc(), in_maps, core_ids=list(range(N_CORES)))
    return np.concatenate([r["out"] for r in res.results], axis=0)


# revision 20
# speedup vs baseline: 1.3827x; 1.2329x over previous
"""Trainium2 Bass kernel for the Flux_Kernels 5-point Dirichlet stencil.

out[i,j] = D*s0*(u[i-1,j] + u[i+1,j] + u[i,j-1] + u[i,j+1]) + 4*D*s1*u[i,j]
with out-of-range neighbors replaced by dirichlet_val[{0,1,2,3}].

Strategy: pad u with the Dirichlet constants into S [4098, 4098] on the host,
shard along rows: core k gets S[512k : 512k+514] (1-row halo each side baked
into the slab). On each core, tiles of 128 consecutive padded rows are
processed with partition p <-> padded row r0+p:
  - TensorE: tridiagonal matmul W.T @ tile -> PSUM[p] = a*up + c*ctr + a*down
    centered at padded row r0+p (rows 0 and 127 are incomplete and discarded)
  - VectorE: lr[p] = tile[p, j] + tile[p, j+2]  (left+right sums)
  - VectorE: o[p] = (lr[p] * a) + PSUM[p]       (fused scalar_tensor_tensor,
    also evacuates PSUM)
  - output DMA stores partitions 1..126 -> 126 output rows per tile; the
    DMA absorbs the one-row shift that compute engines cannot express.
    Stores are issued on the ACT HWDGE ring so they never head-of-line
    block input prefetches on the SP ring.
Consecutive tiles overlap by 2 rows. The 8-row remainder tile is reshaped
into 4 column-blocks of 1024 placed at partition bases {0,32,64,96} so its
vector work engages 128 partitions instead of 10. All scalars (a = D*s0,
c = 4*D*s1, weight matrices) are computed on the host from runtime inputs;
the per-partition coefficient `a` rides as an extra column of w_main.
"""

import sys

import numpy as np

if "/opt/trn_rl_repo" not in sys.path:
    sys.path.insert(0, "/opt/trn_rl_repo")

NX, NY = 4096, 4096
N_CORES = 8
ROWS_PER_CORE = NX // N_CORES          # 512
SLAB_ROWS = ROWS_PER_CORE + 2          # 514
PAD_COLS = NY + 2                      # 4098
TILE_OUT = 126                         # output rows per full tile
FULL_TILES = ROWS_PER_CORE // TILE_OUT  # 4
LAST_OUT = ROWS_PER_CORE - FULL_TILES * TILE_OUT  # 8
LAST_IN = LAST_OUT + 2                 # 10
LAST_R0 = FULL_TILES * TILE_OUT        # 504
PSUM_CHUNK = 2048                      # free-dim columns per PSUM tile
MM_N = 512                             # matmul moving free dim (1 PSUM bank)
BLK = 2048                             # tile-4 column-block width

_CACHE: dict = {}


def _build_nc():
    import concourse.bass as bass
    import concourse.mybir as mybir
    from concourse import bacc
    from concourse.tile import TileContext

    f32 = mybir.dt.float32
    add = mybir.AluOpType.add
    mult = mybir.AluOpType.mult

    nc = bacc.Bacc(None, target_bir_lowering=False)
    s_in = nc.dram_tensor("s_in", (SLAB_ROWS, PAD_COLS), f32, kind="ExternalInput")
    w_main = nc.dram_tensor("w_main", (128, 129), f32, kind="ExternalInput")
    w_aux = nc.dram_tensor("w_aux", (128, LAST_IN), f32, kind="ExternalInput")
    out = nc.dram_tensor("out", (ROWS_PER_CORE, NY), f32, kind="ExternalOutput")

    with TileContext(nc) as tc:
        with (
            tc.tile_pool(name="const", bufs=1) as cpool,
            tc.tile_pool(name="inp", bufs=4) as ipool,
            tc.tile_pool(name="lrp", bufs=2) as lpool,
            tc.tile_pool(name="op", bufs=3) as opool,
            tc.tile_pool(name="psum", bufs=2, space=bass.MemorySpace.PSUM) as ppool,
        ):
            # tile-0 input first (half-split so PE/DVE can start early)
            in_tiles = [
                ipool.tile([128, PAD_COLS], f32, tag="in", name=f"in{i}")
                for i in range(4)
            ]
            nc.sync.dma_start(out=in_tiles[0][:, 0:2050], in_=s_in[0:128, 0:2050])

            w_t = cpool.tile([128, 129], f32)
            nc.sync.dma_start(out=w_t[:], in_=w_main[:])
            w5_t = cpool.tile([128, LAST_IN], f32)
            nc.sync.dma_start(out=w5_t[:], in_=w_aux[:])
            coef = w_t[:, 128:129]

            nc.sync.dma_start(
                out=in_tiles[0][:, 2048:PAD_COLS], in_=s_in[0:128, 2048:PAD_COLS]
            )
            for t in range(1, FULL_TILES):
                nc.sync.dma_start(
                    out=in_tiles[t][:], in_=s_in[TILE_OUT * t : TILE_OUT * t + 128, :]
                )
            # remainder tile: 2 column-blocks at partition bases {0, 64}
            in5 = ipool.tile([128, BLK + 2], f32, tag="in")
            for cb in range(2):
                nc.sync.dma_start(
                    out=in5[64 * cb : 64 * cb + LAST_IN, :],
                    in_=s_in[LAST_R0:SLAB_ROWS, BLK * cb : BLK * cb + BLK + 2],
                )

            for t in range(FULL_TILES):
                r0 = TILE_OUT * t
                in_t = in_tiles[t]

                # left+right sums (one whole-row DVE pass)
                lr_t = lpool.tile([128, NY], f32, tag="lr")
                nc.vector.tensor_add(
                    out=lr_t[:], in0=in_t[:, 0:NY], in1=in_t[:, 2 : NY + 2]
                )

                o_t = opool.tile([128, NY], f32, tag="o")
                for h in range(NY // PSUM_CHUNK):
                    ps = ppool.tile([128, PSUM_CHUNK], f32, tag="ps")
                    for q in range(PSUM_CHUNK // MM_N):
                        cc = h * PSUM_CHUNK + q * MM_N
                        # a*up + c*ctr + a*down (rows via tridiagonal weights)
                        nc.tensor.matmul(
                            ps[:, q * MM_N : (q + 1) * MM_N],
                            w_t[:, 0:128],
                            in_t[:, 1 + cc : 1 + cc + MM_N],
                            start=True,
                            stop=True,
                        )
                    # o = a*(left+right) + psum; also evacuates PSUM
                    nc.vector.scalar_tensor_tensor(
                        out=o_t[:, h * PSUM_CHUNK : (h + 1) * PSUM_CHUNK],
                        in0=lr_t[:, h * PSUM_CHUNK : (h + 1) * PSUM_CHUNK],
                        scalar=coef,
                        in1=ps[:, :],
                        op0=mult,
                        op1=add,
                    )
                    # stores ride the ACT HWDGE ring (SP ring is for loads)
                    nc.scalar.dma_start(
                        out=out[r0 : r0 + TILE_OUT, h * PSUM_CHUNK : (h + 1) * PSUM_CHUNK],
                        in_=o_t[1 : 1 + TILE_OUT, h * PSUM_CHUNK : (h + 1) * PSUM_CHUNK],
                    )

            # remainder tile: partition 32*cb + r  <->  padded row 504+r,
            # columns [1024*cb, 1024*cb + 1025]
            lr5 = lpool.tile([128, BLK], f32, tag="lr")
            nc.vector.tensor_add(
                out=lr5[:], in0=in5[:, 0:BLK], in1=in5[:, 2 : BLK + 2]
            )
            o5 = opool.tile([128, BLK], f32, tag="o")
            ps5 = ppool.tile([128, BLK], f32, tag="ps")
            for q in range(BLK // MM_N):
                for cb in range(2):
                    nc.tensor.matmul(
                        ps5[64 * cb : 64 * cb + LAST_IN, q * MM_N : (q + 1) * MM_N],
                        w5_t[64 * cb : 64 * cb + LAST_IN, :],
                        in5[64 * cb : 64 * cb + LAST_IN, 1 + q * MM_N : 1 + (q + 1) * MM_N],
                        start=True,
                        stop=True,
                    )
            nc.vector.scalar_tensor_tensor(
                out=o5[:, :],
                in0=lr5[:, :],
                scalar=coef,
                in1=ps5[:, :],
                op0=mult,
                op1=add,
            )
            for cb in range(2):
                nc.scalar.dma_start(
                    out=out[LAST_R0 : LAST_R0 + LAST_OUT, BLK * cb : BLK * (cb + 1)],
                    in_=o5[64 * cb + 1 : 64 * cb + 1 + LAST_OUT, :],
                )

    nc.compile()
    return nc


def _get_nc():
    if "nc" not in _CACHE:
        _CACHE["nc"] = _build_nc()
    return _CACHE["nc"]


def _tridiag(n, a, c):
    w = np.zeros((n, n), dtype=np.float32)
    i = np.arange(n)
    w[i, i] = c
    w[i[:-1], i[1:]] = a  # k = m-1 (up neighbor)
    w[i[1:], i[:-1]] = a  # k = m+1 (down neighbor)
    return w


def _weight_inputs(a, c):
    w_main = np.empty((128, 129), dtype=np.float32)
    w_main[:, 0:128] = _tridiag(128, a, c)
    w_main[:, 128] = a  # per-partition STT coefficient
    w_aux = np.zeros((128, LAST_IN), dtype=np.float32)
    for cb in range(2):
        w_aux[64 * cb : 64 * cb + LAST_IN, :] = _tridiag(LAST_IN, a, c)
    return {"w_main": w_main, "w_aux": w_aux}


def kernel(u_main, u_coupled=None, D_eff=None, dirichlet_val=None, stencil=None,
           t=None, **_ignored):
    u = np.asarray(u_main, dtype=np.float32)
    assert u.shape == (NX, NY), u.shape
    D = float(np.asarray(D_eff).reshape(-1)[0])
    st = np.asarray(stencil).reshape(-1)
    s0, s1 = float(st[0]), float(st[1])
    dv = np.asarray(dirichlet_val, dtype=np.float32).reshape(-1)
    a = np.float32(D * s0)
    c = np.float32(4.0 * D * s1)

    S = np.empty((NX + 2, NY + 2), dtype=np.float32)
    S[1:-1, 1:-1] = u
    S[0, :] = dv[0]       # x- boundary (row 0 up-neighbor)
    S[-1, :] = dv[1]      # x+ boundary
    S[1:-1, 0] = dv[2]    # y- boundary
    S[1:-1, -1] = dv[3]   # y+ boundary

    in_maps = [
        {
            "s_in": np.ascontiguousarray(S[ROWS_PER_CORE * k : ROWS_PER_CORE * k + SLAB_ROWS]),
            **_weight_inputs(a, c),
        }
        for k in range(N_CORES)
    ]

    from concourse.bass_utils import run_bass_kernel_spmd

    res = run_bass_kernel_spmd(_get_nc(), in_maps, core_ids=list(range(N_CORES)))
    return np.concatenate([r["out"] for r in res.results], axis=0)


# revision 23
# speedup vs baseline: 1.4714x; 1.0641x over previous
"""Trainium2 Bass kernel for the Flux_Kernels 5-point Dirichlet stencil.

out[i,j] = D*s0*(u[i-1,j] + u[i+1,j] + u[i,j-1] + u[i,j+1]) + 4*D*s1*u[i,j]
with out-of-range neighbors replaced by dirichlet_val[{0,1,2,3}].

Strategy: pad u with the Dirichlet constants into S [4098, 4098] on the host,
shard along rows: core k gets S[512k : 512k+514] (1-row halo each side baked
into the slab). On each core, tiles of 128 consecutive padded rows are
processed with partition p <-> padded row r0+p:
  - TensorE: tridiagonal matmul W.T @ tile -> PSUM[p] = a*up + c*ctr + a*down
    centered at padded row r0+p (rows 0 and 127 are incomplete and discarded)
  - VectorE: lr[p] = tile[p, j] + tile[p, j+2]  (left+right sums)
  - VectorE: o[p] = (lr[p] * a) + PSUM[p]       (fused scalar_tensor_tensor,
    also evacuates PSUM)
  - output DMA stores partitions 1..126 -> 126 output rows per tile; the
    DMA absorbs the one-row shift that compute engines cannot express.
    Stores are issued on the ACT HWDGE ring so they never head-of-line
    block input prefetches on the SP ring.
Consecutive tiles overlap by 2 rows. The 8-row remainder tile is reshaped
into 4 column-blocks of 1024 placed at partition bases {0,32,64,96} so its
vector work engages 128 partitions instead of 10. All scalars (a = D*s0,
c = 4*D*s1, weight matrices) are computed on the host from runtime inputs;
the per-partition coefficient `a` rides as an extra column of w_main.
"""

import sys

import numpy as np

if "/opt/trn_rl_repo" not in sys.path:
    sys.path.insert(0, "/opt/trn_rl_repo")

NX, NY = 4096, 4096
N_CORES = 8
ROWS_PER_CORE = NX // N_CORES          # 512
SLAB_ROWS = ROWS_PER_CORE + 2          # 514
PAD_COLS = NY + 2                      # 4098
TILE_OUT = 126                         # output rows per full tile
FULL_TILES = ROWS_PER_CORE // TILE_OUT  # 4
LAST_OUT = ROWS_PER_CORE - FULL_TILES * TILE_OUT  # 8
LAST_IN = LAST_OUT + 2                 # 10
LAST_R0 = FULL_TILES * TILE_OUT        # 504
PSUM_CHUNK = 2048                      # free-dim columns per PSUM tile
MM_N = 512                             # matmul moving free dim (1 PSUM bank)
BLK = 2048                             # tile-4 column-block width

_CACHE: dict = {}


def _build_nc():
    import concourse.bass as bass
    import concourse.mybir as mybir
    from concourse import bacc
    from concourse.tile import TileContext

    f32 = mybir.dt.float32
    add = mybir.AluOpType.add
    mult = mybir.AluOpType.mult

    nc = bacc.Bacc(None, target_bir_lowering=False)
    s_in = nc.dram_tensor("s_in", (SLAB_ROWS, PAD_COLS), f32, kind="ExternalInput")
    w_main = nc.dram_tensor("w_main", (128, 129), f32, kind="ExternalInput")
    w_aux = nc.dram_tensor("w_aux", (128, LAST_IN), f32, kind="ExternalInput")
    out = nc.dram_tensor("out", (ROWS_PER_CORE, NY), f32, kind="ExternalOutput")

    with TileContext(nc) as tc:
        with (
            tc.tile_pool(name="const", bufs=1) as cpool,
            tc.tile_pool(name="inp", bufs=4) as ipool,
            tc.tile_pool(name="lrp", bufs=1) as lpool,
            tc.tile_pool(name="op", bufs=3) as opool,
            tc.tile_pool(name="psum", bufs=2, space=bass.MemorySpace.PSUM) as ppool,
        ):
            # tile-0 input first (half-split so PE/DVE can start early)
            in_tiles = [
                ipool.tile([128, PAD_COLS], f32, tag="in", name=f"in{i}")
                for i in range(4)
            ]
            nc.sync.dma_start(out=in_tiles[0][:, 0:2050], in_=s_in[0:128, 0:2050])

            w_t = cpool.tile([128, 129], f32)
            nc.sync.dma_start(out=w_t[:], in_=w_main[:])
            w5_t = cpool.tile([128, LAST_IN], f32)
            nc.sync.dma_start(out=w5_t[:], in_=w_aux[:])
            coef = w_t[:, 128:129]

            nc.sync.dma_start(
                out=in_tiles[0][:, 2048:PAD_COLS], in_=s_in[0:128, 2048:PAD_COLS]
            )
            for t in range(1, FULL_TILES):
                nc.sync.dma_start(
                    out=in_tiles[t][:], in_=s_in[TILE_OUT * t : TILE_OUT * t + 128, :]
                )
            # remainder tile: 2 column-blocks at partition bases {0, 64}
            in5 = ipool.tile([128, BLK + 2], f32, tag="in")
            for cb in range(2):
                nc.sync.dma_start(
                    out=in5[64 * cb : 64 * cb + LAST_IN, :],
                    in_=s_in[LAST_R0:SLAB_ROWS, BLK * cb : BLK * cb + BLK + 2],
                )

            for t in range(FULL_TILES):
                r0 = TILE_OUT * t
                in_t = in_tiles[t]

                # left+right sums. Tiles 0-2 on DVE (bufs=1 keeps the DVE
                # stream strictly tile-ordered); tile 3 on the otherwise-idle
                # GpSimd, well off the critical path.
                if t < 3:
                    lr_t = lpool.tile([128, NY], f32, tag="lr")
                    nc.vector.tensor_add(
                        out=lr_t[:], in0=in_t[:, 0:NY], in1=in_t[:, 2 : NY + 2]
                    )
                else:
                    lr_t = lpool.tile([128, NY], f32, tag="lrg")
                    nc.gpsimd.tensor_add(
                        out=lr_t[:], in0=in_t[:, 0:NY], in1=in_t[:, 2 : NY + 2]
                    )

                o_t = opool.tile([128, NY], f32, tag="o")
                for h in range(NY // PSUM_CHUNK):
                    ps = ppool.tile([128, PSUM_CHUNK], f32, tag="ps")
                    for q in range(PSUM_CHUNK // MM_N):
                        cc = h * PSUM_CHUNK + q * MM_N
                        # a*up + c*ctr + a*down (rows via tridiagonal weights)
                        nc.tensor.matmul(
                            ps[:, q * MM_N : (q + 1) * MM_N],
                            w_t[:, 0:128],
                            in_t[:, 1 + cc : 1 + cc + MM_N],
                            start=True,
                            stop=True,
                        )
                    # o = a*(left+right) + psum; also evacuates PSUM
                    nc.vector.scalar_tensor_tensor(
                        out=o_t[:, h * PSUM_CHUNK : (h + 1) * PSUM_CHUNK],
                        in0=lr_t[:, h * PSUM_CHUNK : (h + 1) * PSUM_CHUNK],
                        scalar=coef,
                        in1=ps[:, :],
                        op0=mult,
                        op1=add,
                    )
                    # stores ride the ACT HWDGE ring (SP ring is for loads)
                    nc.scalar.dma_start(
                        out=out[r0 : r0 + TILE_OUT, h * PSUM_CHUNK : (h + 1) * PSUM_CHUNK],
                        in_=o_t[1 : 1 + TILE_OUT, h * PSUM_CHUNK : (h + 1) * PSUM_CHUNK],
                    )

            # remainder tile: partition 32*cb + r  <->  padded row 504+r,
            # columns [1024*cb, 1024*cb + 1025]
            lr5 = lpool.tile([128, BLK], f32, tag="lrg5")
            nc.gpsimd.tensor_add(
                out=lr5[:], in0=in5[:, 0:BLK], in1=in5[:, 2 : BLK + 2]
            )
            o5 = opool.tile([128, BLK], f32, tag="o")
            ps5 = ppool.tile([128, BLK], f32, tag="ps")
            for q in range(BLK // MM_N):
                for cb in range(2):
                    nc.tensor.matmul(
                        ps5[64 * cb : 64 * cb + LAST_IN, q * MM_N : (q + 1) * MM_N],
                        w5_t[64 * cb : 64 * cb + LAST_IN, :],
                        in5[64 * cb : 64 * cb + LAST_IN, 1 + q * MM_N : 1 + (q + 1) * MM_N],
                        start=True,
                        stop=True,
                    )
            nc.vector.scalar_tensor_tensor(
                out=o5[:, :],
                in0=lr5[:, :],
                scalar=coef,
                in1=ps5[:, :],
                op0=mult,
                op1=add,
            )
            for cb in range(2):
                nc.scalar.dma_start(
                    out=out[LAST_R0 : LAST_R0 + LAST_OUT, BLK * cb : BLK * (cb + 1)],
                    in_=o5[64 * cb + 1 : 64 * cb + 1 + LAST_OUT, :],
                )

    nc.compile()
    return nc


def _get_nc():
    if "nc" not in _CACHE:
        _CACHE["nc"] = _build_nc()
    return _CACHE["nc"]


def _tridiag(n, a, c):
    w = np.zeros((n, n), dtype=np.float32)
    i = np.arange(n)
    w[i, i] = c
    w[i[:-1], i[1:]] = a  # k = m-1 (up neighbor)
    w[i[1:], i[:-1]] = a  # k = m+1 (down neighbor)
    return w


def _weight_inputs(a, c):
    w_main = np.empty((128, 129), dtype=np.float32)
    w_main[:, 0:128] = _tridiag(128, a, c)
    w_main[:, 128] = a  # per-partition STT coefficient
    w_aux = np.zeros((128, LAST_IN), dtype=np.float32)
    for cb in range(2):
        w_aux[64 * cb : 64 * cb + LAST_IN, :] = _tridiag(LAST_IN, a, c)
    return {"w_main": w_main, "w_aux": w_aux}


def kernel(u_main, u_coupled=None, D_eff=None, dirichlet_val=None, stencil=None,
           t=None, **_ignored):
    u = np.asarray(u_main, dtype=np.float32)
    assert u.shape == (NX, NY), u.shape
    D = float(np.asarray(D_eff).reshape(-1)[0])
    st = np.asarray(stencil).reshape(-1)
    s0, s1 = float(st[0]), float(st[1])
    dv = np.asarray(dirichlet_val, dtype=np.float32).reshape(-1)
    a = np.float32(D * s0)
    c = np.float32(4.0 * D * s1)

    S = np.empty((NX + 2, NY + 2), dtype=np.float32)
    S[1:-1, 1:-1] = u
    S[0, :] = dv[0]       # x- boundary (row 0 up-neighbor)
    S[-1, :] = dv[1]      # x+ boundary
    S[1:-1, 0] = dv[2]    # y- boundary
    S[1:-1, -1] = dv[3]   # y+ boundary

    in_maps = [
        {
            "s_in": np.ascontiguousarray(S[ROWS_PER_CORE * k : ROWS_PER_CORE * k + SLAB_ROWS]),
            **_weight_inputs(a, c),
        }
        for k in range(N_CORES)
    ]

    from concourse.bass_utils import run_bass_kernel_spmd

    res = run_bass_kernel_spmd(_get_nc(), in_maps, core_ids=list(range(N_CORES)))
    return np.concatenate([r["out"] for r in res.results], axis=0)


# revision 26
# speedup vs baseline: 1.5044x; 1.0224x over previous
"""Trainium2 Bass kernel for the Flux_Kernels 5-point Dirichlet stencil.

out[i,j] = D*s0*(u[i-1,j] + u[i+1,j] + u[i,j-1] + u[i,j+1]) + 4*D*s1*u[i,j]
with out-of-range neighbors replaced by dirichlet_val[{0,1,2,3}].

Strategy: pad u with the Dirichlet constants into S [4098, 4098] on the host,
shard along rows: core k gets S[512k : 512k+514] (1-row halo each side baked
into the slab). On each core, tiles of 128 consecutive padded rows are
processed with partition p <-> padded row r0+p:
  - TensorE: tridiagonal matmul W.T @ tile -> PSUM[p] = a*up + c*ctr + a*down
    centered at padded row r0+p (rows 0 and 127 are incomplete and discarded)
  - VectorE: lr[p] = tile[p, j] + tile[p, j+2]  (left+right sums)
  - VectorE: o[p] = (lr[p] * a) + PSUM[p]       (fused scalar_tensor_tensor,
    also evacuates PSUM)
  - output DMA stores partitions 1..126 -> 126 output rows per tile; the
    DMA absorbs the one-row shift that compute engines cannot express.
    Stores are issued on the ACT HWDGE ring so they never head-of-line
    block input prefetches on the SP ring.
Consecutive tiles overlap by 2 rows. The 8-row remainder tile is reshaped
into 4 column-blocks of 1024 placed at partition bases {0,32,64,96} so its
vector work engages 128 partitions instead of 10. All scalars (a = D*s0,
c = 4*D*s1, weight matrices) are computed on the host from runtime inputs;
the per-partition coefficient `a` rides as an extra column of w_main.
"""

import sys

import numpy as np

if "/opt/trn_rl_repo" not in sys.path:
    sys.path.insert(0, "/opt/trn_rl_repo")

NX, NY = 4096, 4096
N_CORES = 8
ROWS_PER_CORE = NX // N_CORES          # 512
SLAB_ROWS = ROWS_PER_CORE + 2          # 514
PAD_COLS = NY + 2                      # 4098
TILE_OUT = 126                         # output rows per full tile
FULL_TILES = ROWS_PER_CORE // TILE_OUT  # 4
LAST_OUT = ROWS_PER_CORE - FULL_TILES * TILE_OUT  # 8
LAST_IN = LAST_OUT + 2                 # 10
LAST_R0 = FULL_TILES * TILE_OUT        # 504
PSUM_CHUNK = 2048                      # free-dim columns per PSUM tile
MM_N = 512                             # matmul moving free dim (1 PSUM bank)
BLK = 2048                             # tile-4 column-block width

_CACHE: dict = {}


def _build_nc():
    import concourse.bass as bass
    import concourse.mybir as mybir
    from concourse import bacc
    from concourse.tile import TileContext

    f32 = mybir.dt.float32
    add = mybir.AluOpType.add
    mult = mybir.AluOpType.mult

    nc = bacc.Bacc(None, target_bir_lowering=False)
    s_in = nc.dram_tensor("s_in", (SLAB_ROWS, PAD_COLS), f32, kind="ExternalInput")
    w_main = nc.dram_tensor("w_main", (128, 129), f32, kind="ExternalInput")
    w_aux = nc.dram_tensor("w_aux", (128, LAST_IN), f32, kind="ExternalInput")
    out = nc.dram_tensor("out", (ROWS_PER_CORE, NY), f32, kind="ExternalOutput")

    with TileContext(nc) as tc:
        with (
            tc.tile_pool(name="const", bufs=1) as cpool,
            tc.tile_pool(name="inp", bufs=4) as ipool,
            tc.tile_pool(name="lrp", bufs=1) as lpool,
            tc.tile_pool(name="lrg", bufs=2) as gpool,
            tc.tile_pool(name="op", bufs=3) as opool,
            tc.tile_pool(name="psum", bufs=2, space=bass.MemorySpace.PSUM) as ppool,
        ):
            # tile-0 input first (half-split so PE/DVE can start early)
            in_tiles = [
                ipool.tile([128, PAD_COLS], f32, tag="in", name=f"in{i}")
                for i in range(4)
            ]
            nc.sync.dma_start(out=in_tiles[0][:, 0:2050], in_=s_in[0:128, 0:2050])

            w_t = cpool.tile([128, 129], f32)
            nc.sync.dma_start(out=w_t[:], in_=w_main[:])
            w5_t = cpool.tile([128, LAST_IN], f32)
            nc.sync.dma_start(out=w5_t[:], in_=w_aux[:])
            coef = w_t[:, 128:129]

            nc.sync.dma_start(
                out=in_tiles[0][:, 2048:PAD_COLS], in_=s_in[0:128, 2048:PAD_COLS]
            )
            for t in range(1, FULL_TILES):
                nc.sync.dma_start(
                    out=in_tiles[t][:], in_=s_in[TILE_OUT * t : TILE_OUT * t + 128, :]
                )
            # remainder tile: 2 column-blocks at partition bases {0, 64}
            in5 = ipool.tile([128, BLK + 2], f32, tag="in")
            for cb in range(2):
                nc.sync.dma_start(
                    out=in5[64 * cb : 64 * cb + LAST_IN, :],
                    in_=s_in[LAST_R0:SLAB_ROWS, BLK * cb : BLK * cb + BLK + 2],
                )

            for t in range(FULL_TILES):
                r0 = TILE_OUT * t
                in_t = in_tiles[t]

                # left+right sums. Tiles 0-1 on DVE (bufs=1 keeps the DVE
                # stream strictly tile-ordered); tiles 2-3 on the otherwise
                # idle GpSimd, off the critical path (its 2-src adds contend
                # with DVE 2-src adds but not with the STT combines).
                if t < 2:
                    lr_t = lpool.tile([128, NY], f32, tag="lr")
                    nc.vector.tensor_add(
                        out=lr_t[:], in0=in_t[:, 0:NY], in1=in_t[:, 2 : NY + 2]
                    )
                else:
                    lr_t = gpool.tile([128, NY], f32, tag="lrg")
                    nc.gpsimd.tensor_add(
                        out=lr_t[:], in0=in_t[:, 0:NY], in1=in_t[:, 2 : NY + 2]
                    )

                o_t = opool.tile([128, NY], f32, tag="o")
                for h in range(NY // PSUM_CHUNK):
                    ps = ppool.tile([128, PSUM_CHUNK], f32, tag="ps")
                    for q in range(PSUM_CHUNK // MM_N):
                        cc = h * PSUM_CHUNK + q * MM_N
                        # a*up + c*ctr + a*down (rows via tridiagonal weights)
                        nc.tensor.matmul(
                            ps[:, q * MM_N : (q + 1) * MM_N],
                            w_t[:, 0:128],
                            in_t[:, 1 + cc : 1 + cc + MM_N],
                            start=True,
                            stop=True,
                        )
                    # o = a*(left+right) + psum; also evacuates PSUM
                    nc.vector.scalar_tensor_tensor(
                        out=o_t[:, h * PSUM_CHUNK : (h + 1) * PSUM_CHUNK],
                        in0=lr_t[:, h * PSUM_CHUNK : (h + 1) * PSUM_CHUNK],
                        scalar=coef,
                        in1=ps[:, :],
                        op0=mult,
                        op1=add,
                    )
                    # stores ride the ACT HWDGE ring (SP ring is for loads)
                    nc.scalar.dma_start(
                        out=out[r0 : r0 + TILE_OUT, h * PSUM_CHUNK : (h + 1) * PSUM_CHUNK],
                        in_=o_t[1 : 1 + TILE_OUT, h * PSUM_CHUNK : (h + 1) * PSUM_CHUNK],
                    )

            # remainder tile: partition 32*cb + r  <->  padded row 504+r,
            # columns [1024*cb, 1024*cb + 1025]
            lr5 = gpool.tile([128, BLK], f32, tag="lrg5")
            nc.gpsimd.tensor_add(
                out=lr5[:], in0=in5[:, 0:BLK], in1=in5[:, 2 : BLK + 2]
            )
            o5 = opool.tile([128, BLK], f32, tag="o")
            ps5 = ppool.tile([128, BLK], f32, tag="ps")
            for q in range(BLK // MM_N):
                for cb in range(2):
                    nc.tensor.matmul(
                        ps5[64 * cb : 64 * cb + LAST_IN, q * MM_N : (q + 1) * MM_N],
                        w5_t[64 * cb : 64 * cb + LAST_IN, :],
                        in5[64 * cb : 64 * cb + LAST_IN, 1 + q * MM_N : 1 + (q + 1) * MM_N],
                        start=True,
                        stop=True,
                    )
            nc.vector.scalar_tensor_tensor(
                out=o5[:, :],
                in0=lr5[:, :],
                scalar=coef,
                in1=ps5[:, :],
                op0=mult,
                op1=add,
            )
            for cb in range(2):
                nc.scalar.dma_start(
                    out=out[LAST_R0 : LAST_R0 + LAST_OUT, BLK * cb : BLK * (cb + 1)],
                    in_=o5[64 * cb + 1 : 64 * cb + 1 + LAST_OUT, :],
                )

    nc.compile()
    return nc


def _get_nc():
    if "nc" not in _CACHE:
        _CACHE["nc"] = _build_nc()
    return _CACHE["nc"]


def _tridiag(n, a, c):
    w = np.zeros((n, n), dtype=np.float32)
    i = np.arange(n)
    w[i, i] = c
    w[i[:-1], i[1:]] = a  # k = m-1 (up neighbor)
    w[i[1:], i[:-1]] = a  # k = m+1 (down neighbor)
    return w


def _weight_inputs(a, c):
    w_main = np.empty((128, 129), dtype=np.float32)
    w_main[:, 0:128] = _tridiag(128, a, c)
    w_main[:, 128] = a  # per-partition STT coefficient
    w_aux = np.zeros((128, LAST_IN), dtype=np.float32)
    for cb in range(2):
        w_aux[64 * cb : 64 * cb + LAST_IN, :] = _tridiag(LAST_IN, a, c)
    return {"w_main": w_main, "w_aux": w_aux}


def kernel(u_main, u_coupled=None, D_eff=None, dirichlet_val=None, stencil=None,
           t=None, **_ignored):
    u = np.asarray(u_main, dtype=np.float32)
    assert u.shape == (NX, NY), u.shape
    D = float(np.asarray(D_eff).reshape(-1)[0])
    st = np.asarray(stencil).reshape(-1)
    s0, s1 = float(st[0]), float(st[1])
    dv = np.asarray(dirichlet_val, dtype=np.float32).reshape(-1)
    a = np.float32(D * s0)
    c = np.float32(4.0 * D * s1)

    S = np.empty((NX + 2, NY + 2), dtype=np.float32)
    S[1:-1, 1:-1] = u
    S[0, :] = dv[0]       # x- boundary (row 0 up-neighbor)
    S[-1, :] = dv[1]      # x+ boundary
    S[1:-1, 0] = dv[2]    # y- boundary
    S[1:-1, -1] = dv[3]   # y+ boundary

    in_maps = [
        {
            "s_in": np.ascontiguousarray(S[ROWS_PER_CORE * k : ROWS_PER_CORE * k + SLAB_ROWS]),
            **_weight_inputs(a, c),
        }
        for k in range(N_CORES)
    ]

    from concourse.bass_utils import run_bass_kernel_spmd

    res = run_bass_kernel_spmd(_get_nc(), in_maps, core_ids=list(range(N_CORES)))
    return np.concatenate([r["out"] for r in res.results], axis=0)


# revision 27
# speedup vs baseline: 1.5975x; 1.0619x over previous
"""Trainium2 Bass kernel for the Flux_Kernels 5-point Dirichlet stencil.

out[i,j] = D*s0*(u[i-1,j] + u[i+1,j] + u[i,j-1] + u[i,j+1]) + 4*D*s1*u[i,j]
with out-of-range neighbors replaced by dirichlet_val[{0,1,2,3}].

Strategy: pad u with the Dirichlet constants into S [4098, 4098] on the host,
shard along rows: core k gets S[512k : 512k+514] (1-row halo each side baked
into the slab). On each core, tiles of 128 consecutive padded rows are
processed with partition p <-> padded row r0+p:
  - TensorE: tridiagonal matmul W.T @ tile -> PSUM[p] = a*up + c*ctr + a*down
    centered at padded row r0+p (rows 0 and 127 are incomplete and discarded)
  - VectorE: lr[p] = tile[p, j] + tile[p, j+2]  (left+right sums)
  - VectorE: o[p] = (lr[p] * a) + PSUM[p]       (fused scalar_tensor_tensor,
    also evacuates PSUM)
  - output DMA stores partitions 1..126 -> 126 output rows per tile; the
    DMA absorbs the one-row shift that compute engines cannot express.
    Stores are issued on the ACT HWDGE ring so they never head-of-line
    block input prefetches on the SP ring.
Consecutive tiles overlap by 2 rows. The 8-row remainder tile is reshaped
into 4 column-blocks of 1024 placed at partition bases {0,32,64,96} so its
vector work engages 128 partitions instead of 10. All scalars (a = D*s0,
c = 4*D*s1, weight matrices) are computed on the host from runtime inputs;
the per-partition coefficient `a` rides as an extra column of w_main.
"""

import sys

import numpy as np

if "/opt/trn_rl_repo" not in sys.path:
    sys.path.insert(0, "/opt/trn_rl_repo")

NX, NY = 4096, 4096
N_CORES = 8
ROWS_PER_CORE = NX // N_CORES          # 512
SLAB_ROWS = ROWS_PER_CORE + 2          # 514
PAD_COLS = NY + 2                      # 4098
TILE_OUT = 126                         # output rows per full tile
FULL_TILES = ROWS_PER_CORE // TILE_OUT  # 4
LAST_OUT = ROWS_PER_CORE - FULL_TILES * TILE_OUT  # 8
LAST_IN = LAST_OUT + 2                 # 10
LAST_R0 = FULL_TILES * TILE_OUT        # 504
PSUM_CHUNK = 2048                      # free-dim columns per PSUM tile
MM_N = 512                             # matmul moving free dim (1 PSUM bank)
BLK = 2048                             # tile-4 column-block width

_CACHE: dict = {}


def _build_nc():
    import concourse.bass as bass
    import concourse.mybir as mybir
    from concourse import bacc
    from concourse.tile import TileContext

    f32 = mybir.dt.float32
    add = mybir.AluOpType.add
    mult = mybir.AluOpType.mult

    nc = bacc.Bacc(None, target_bir_lowering=False)
    s_in = nc.dram_tensor("s_in", (SLAB_ROWS, PAD_COLS), f32, kind="ExternalInput")
    w_main = nc.dram_tensor("w_main", (128, 129), f32, kind="ExternalInput")
    w_aux = nc.dram_tensor("w_aux", (128, LAST_IN), f32, kind="ExternalInput")
    out = nc.dram_tensor("out", (ROWS_PER_CORE, NY), f32, kind="ExternalOutput")

    with TileContext(nc) as tc:
        with (
            tc.tile_pool(name="const", bufs=1) as cpool,
            tc.tile_pool(name="inp", bufs=4) as ipool,
            tc.tile_pool(name="lrp", bufs=1) as lpool,
            tc.tile_pool(name="lrg", bufs=2) as gpool,
            tc.tile_pool(name="op", bufs=3) as opool,
            tc.tile_pool(name="psum", bufs=2, space=bass.MemorySpace.PSUM) as ppool,
        ):
            in_tiles = [
                ipool.tile([128, PAD_COLS], f32, tag="in", name=f"in{i}")
                for i in range(4)
            ]
            # tile-0 input first so DVE can start the moment it boots
            nc.sync.dma_start(out=in_tiles[0][:], in_=s_in[0:128, :])

            w_t = cpool.tile([128, 129], f32)
            nc.sync.dma_start(out=w_t[:], in_=w_main[:])
            w5_t = cpool.tile([128, LAST_IN], f32)
            nc.sync.dma_start(out=w5_t[:], in_=w_aux[:])
            coef = w_t[:, 128:129]

            nc.sync.dma_start(out=in_tiles[1][:], in_=s_in[TILE_OUT : TILE_OUT + 128, :])
            nc.sync.dma_start(
                out=in_tiles[2][:], in_=s_in[2 * TILE_OUT : 2 * TILE_OUT + 128, :]
            )
            # remainder tile early: 2 column-blocks at partition bases {0, 64}
            in5 = ipool.tile([128, BLK + 2], f32, tag="in5", bufs=1)
            for cb in range(2):
                nc.sync.dma_start(
                    out=in5[64 * cb : 64 * cb + LAST_IN, :],
                    in_=s_in[LAST_R0:SLAB_ROWS, BLK * cb : BLK * cb + BLK + 2],
                )
            nc.sync.dma_start(
                out=in_tiles[3][:], in_=s_in[3 * TILE_OUT : 3 * TILE_OUT + 128, :]
            )

            def full_tile(t):
                r0 = TILE_OUT * t
                in_t = in_tiles[t]
                # left+right sums. Tiles 0-1 on DVE (bufs=1 keeps the DVE
                # stream strictly tile-ordered); tiles 2-3 on the otherwise
                # idle GpSimd, off the critical path (its 2-src adds contend
                # with DVE 2-src adds but not with the STT combines).
                if t < 2:
                    lr_t = lpool.tile([128, NY], f32, tag="lr", name=f"lr{t}")
                    nc.vector.tensor_add(
                        out=lr_t[:], in0=in_t[:, 0:NY], in1=in_t[:, 2 : NY + 2]
                    )
                else:
                    lr_t = gpool.tile([128, NY], f32, tag="lrg", name=f"lrg{t}")
                    nc.gpsimd.tensor_add(
                        out=lr_t[:], in0=in_t[:, 0:NY], in1=in_t[:, 2 : NY + 2]
                    )

                o_t = opool.tile([128, NY], f32, tag="o", name=f"o{t}")
                for h in range(NY // PSUM_CHUNK):
                    ps = ppool.tile([128, PSUM_CHUNK], f32, tag="ps", name=f"ps{t}{h}")
                    for q in range(PSUM_CHUNK // MM_N):
                        cc = h * PSUM_CHUNK + q * MM_N
                        # a*up + c*ctr + a*down (rows via tridiagonal weights)
                        nc.tensor.matmul(
                            ps[:, q * MM_N : (q + 1) * MM_N],
                            w_t[:, 0:128],
                            in_t[:, 1 + cc : 1 + cc + MM_N],
                            start=True,
                            stop=True,
                        )
                    # o = a*(left+right) + psum; also evacuates PSUM
                    nc.vector.scalar_tensor_tensor(
                        out=o_t[:, h * PSUM_CHUNK : (h + 1) * PSUM_CHUNK],
                        in0=lr_t[:, h * PSUM_CHUNK : (h + 1) * PSUM_CHUNK],
                        scalar=coef,
                        in1=ps[:, :],
                        op0=mult,
                        op1=add,
                    )
                    # stores ride the ACT HWDGE ring (SP ring is for loads)
                    nc.scalar.dma_start(
                        out=out[r0 : r0 + TILE_OUT, h * PSUM_CHUNK : (h + 1) * PSUM_CHUNK],
                        in_=o_t[1 : 1 + TILE_OUT, h * PSUM_CHUNK : (h + 1) * PSUM_CHUNK],
                    )

            def last_tile():
                # partition 64*cb + r <-> padded row 504+r,
                # columns [2048*cb, 2048*cb + 2049]
                lr5 = gpool.tile([128, BLK], f32, tag="lrg5", bufs=1)
                nc.gpsimd.tensor_add(
                    out=lr5[:], in0=in5[:, 0:BLK], in1=in5[:, 2 : BLK + 2]
                )
                o5 = opool.tile([128, BLK], f32, tag="o")
                ps5 = ppool.tile([128, BLK], f32, tag="ps")
                for q in range(BLK // MM_N):
                    for cb in range(2):
                        nc.tensor.matmul(
                            ps5[64 * cb : 64 * cb + LAST_IN, q * MM_N : (q + 1) * MM_N],
                            w5_t[64 * cb : 64 * cb + LAST_IN, :],
                            in5[64 * cb : 64 * cb + LAST_IN, 1 + q * MM_N : 1 + (q + 1) * MM_N],
                            start=True,
                            stop=True,
                        )
                nc.vector.scalar_tensor_tensor(
                    out=o5[:, :],
                    in0=lr5[:, :],
                    scalar=coef,
                    in1=ps5[:, :],
                    op0=mult,
                    op1=add,
                )
                for cb in range(2):
                    nc.scalar.dma_start(
                        out=out[LAST_R0 : LAST_R0 + LAST_OUT, BLK * cb : BLK * (cb + 1)],
                        in_=o5[64 * cb + 1 : 64 * cb + 1 + LAST_OUT, :],
                    )

            # remainder tile between t2 and t3 pulls the serial tail forward
            full_tile(0)
            full_tile(1)
            full_tile(2)
            last_tile()
            full_tile(3)

    nc.compile()
    return nc


def _get_nc():
    if "nc" not in _CACHE:
        _CACHE["nc"] = _build_nc()
    return _CACHE["nc"]


def _tridiag(n, a, c):
    w = np.zeros((n, n), dtype=np.float32)
    i = np.arange(n)
    w[i, i] = c
    w[i[:-1], i[1:]] = a  # k = m-1 (up neighbor)
    w[i[1:], i[:-1]] = a  # k = m+1 (down neighbor)
    return w


def _weight_inputs(a, c):
    w_main = np.empty((128, 129), dtype=np.float32)
    w_main[:, 0:128] = _tridiag(128, a, c)
    w_main[:, 128] = a  # per-partition STT coefficient
    w_aux = np.zeros((128, LAST_IN), dtype=np.float32)
    for cb in range(2):
        w_aux[64 * cb : 64 * cb + LAST_IN, :] = _tridiag(LAST_IN, a, c)
    return {"w_main": w_main, "w_aux": w_aux}


def kernel(u_main, u_coupled=None, D_eff=None, dirichlet_val=None, stencil=None,
           t=None, **_ignored):
    u = np.asarray(u_main, dtype=np.float32)
    assert u.shape == (NX, NY), u.shape
    D = float(np.asarray(D_eff).reshape(-1)[0])
    st = np.asarray(stencil).reshape(-1)
    s0, s1 = float(st[0]), float(st[1])
    dv = np.asarray(dirichlet_val, dtype=np.float32).reshape(-1)
    a = np.float32(D * s0)
    c = np.float32(4.0 * D * s1)

    S = np.empty((NX + 2, NY + 2), dtype=np.float32)
    S[1:-1, 1:-1] = u
    S[0, :] = dv[0]       # x- boundary (row 0 up-neighbor)
    S[-1, :] = dv[1]      # x+ boundary
    S[1:-1, 0] = dv[2]    # y- boundary
    S[1:-1, -1] = dv[3]   # y+ boundary

    in_maps = [
        {
            "s_in": np.ascontiguousarray(S[ROWS_PER_CORE * k : ROWS_PER_CORE * k + SLAB_ROWS]),
            **_weight_inputs(a, c),
        }
        for k in range(N_CORES)
    ]

    from concourse.bass_utils import run_bass_kernel_spmd

    res = run_bass_kernel_spmd(_get_nc(), in_maps, core_ids=list(range(N_CORES)))
    return np.concatenate([r["out"] for r in res.results], axis=0)
